# revision 17
# baseline (speedup 1.0000x reference)
"""Multi-head attention (B=2, L=2048, d_model=1024, 16 heads) on 8 TRN2 NeuronCores.

Sharding: data-parallel on batch (2) x tensor-parallel on heads (4 groups of 4
heads). Core c handles batch c//4, head group c%4 (Wq/Wk/Wv column-parallel,
Wo row-parallel). Each core emits a partial (2048, 1024) output projection;
the host sums the 4 partials per batch and adds the bias.

Masked keys contribute exactly zero to the reference output (softmax over
-inf), so each data shard compacts K/V to the kept keys (host-side gather,
padded to a multiple of 512; pad slots get zero V rows and ~0 softmax weight).

Device-side single-pass pipeline, paced by ScalarE exp (the hard floor:
8.4M exps/core at 1 elem/cycle/lane):
  K-proj + Q-proj(win0) first so scores start ~10us in; V-proj and the
  remaining Q windows stream inside the window loop as PE filler work.
  Per 512-query window, per head-pair, per 128-key block:
    S^T = K_h Q_h^T (two heads on disjoint PE row groups, concurrent)
    exp on ScalarE (PSUM->SBUF f16), U^T += [V_h | keep]^T @ expS^T (M=65,
    row 64 = softmax denominator).
  Normalize: denominator rows collect onto partitions 0..3 (gpsimd DMA),
  f32 cast + reciprocal_approx_fast + f16 cast on [4,512] (DVE, ~1.4us/win),
  gpsimd partition_broadcast, f16 DVE multiplies -> ctx^T; folded out-proj
  per window -> DMA out. All PSUM evacuations live on DVE; ScalarE runs
  exp only.
"""

import os
import sys

import numpy as np

for _p in ("/opt/trn_rl_repo", "/root/.axon_site/_ro/trn_rl_repo"):
    if os.path.isdir(_p) and _p not in sys.path:
        sys.path.insert(0, _p)

import concourse.bass as bass  # noqa: E402
import concourse.mybir as mybir  # noqa: E402
import concourse.tile as tile  # noqa: E402
from concourse import bacc  # noqa: E402
from concourse import bass_utils  # noqa: E402
from concourse import library_config  # noqa: E402
from concourse.bass_interp import get_hw_module  # noqa: E402

P = 128
D = 1024          # d_model
LQ = 2048         # query length
DH = 256          # per-core head-group width (4 heads x 64)
HD = 64           # head dim
KC = D // P       # contraction chunks for the projections
MC = DH // P      # 2 partition chunks of the head-group dim
NQW = LQ // 512   # query windows
SCALE = 0.125     # 1/sqrt(HD)
F32 = mybir.dt.float32
F16 = mybir.dt.float16
F8 = mybir.dt.float8e4
H4 = 68  # per-head stride in v_sb: 64 ctx + 1 keep + pad to a 16B-aligned k-tile stride
EXP = mybir.ActivationFunctionType.Exp
NCORES = 8
PAD_KEEP = 0.0  # pad keys: V rows zero, keep 0 -> no denominator contribution

_NC_CACHE: dict[int, object] = {}
LAST_RESULTS = None  # test harness reads exec_time_ns off this
TRACE = bool(int(os.environ.get("KERNEL_TRACE", "0")))


def _ensure_ntff_hook():
    """Provide antenv.axon_hooks when the image lacks it (trace-only path)."""
    import importlib
    import types

    try:
        importlib.import_module("antenv.axon_hooks")
        return
    except ImportError:
        pass
    try:
        import antenv
        from trn_agent_boot.trn_boot import _ntff_profile_via_ctypes
    except ImportError:
        return
    mod = types.ModuleType("antenv.axon_hooks")
    state = {"h": None}
    mod.set_axon_ntff_profile_hook = lambda h: state.__setitem__("h", h)
    mod.get_axon_ntff_profile_hook = lambda: state["h"]
    sys.modules["antenv.axon_hooks"] = mod
    antenv.axon_hooks = mod
    so = "/opt/axon/libaxon_pjrt.so"
    if os.path.exists(so):
        mod.set_axon_ntff_profile_hook(_ntff_profile_via_ctypes(so))


def _build(Lkp: int):
    assert Lkp % 512 == 0
    LKC = Lkp // P          # 128-key blocks
    NKW = Lkp // 512        # 512-key windows
    NPAIR = LKC // 2        # DoubleRow key-block pairs
    nc = bacc.Bacc(
        "TRN2",
        target_bir_lowering=False,
        debug=False,
        enable_asserts=False,
        num_devices=NCORES,
    )

    xq_d = nc.dram_tensor("xq_t", [NQW, P, KC, 512], F16, kind="ExternalInput")
    xk_d = nc.dram_tensor("xk_t", [NKW, P, KC, 512], F16, kind="ExternalInput")
    xv_d = nc.dram_tensor("xv_t", [LKC, P, KC, P], F16, kind="ExternalInput")
    keep_d = nc.dram_tensor("keep", [P, LKC], F16, kind="ExternalInput")
    wq_d = nc.dram_tensor("wq_t", [P, KC, DH], F16, kind="ExternalInput")
    wk_d = nc.dram_tensor("wk_t", [P, KC, DH], F16, kind="ExternalInput")
    wv_d = nc.dram_tensor("wv_t", [P, KC, DH], F16, kind="ExternalInput")
    wo_d = nc.dram_tensor("wo_t", [P, MC, D], F16, kind="ExternalInput")
    out_d = nc.dram_tensor("outp", [LQ, D], F16, kind="ExternalOutput")

    with tile.TileContext(nc) as tc, nc.allow_low_precision(
        reason="f16 PE matmuls; softmax weights are <=1 and averaged over ~1k keys"
    ), tc.tile_pool(name="persist", bufs=1) as pp:
        # ---------------- persistent SBUF ----------------
        wq_sb = pp.tile([P, KC, DH], F16, tag="wq_sb", name="wq_sb")
        wk_sb = pp.tile([P, KC, DH], F16, tag="wk_sb", name="wk_sb")
        wv_sb = pp.tile([P, KC, DH], F16, tag="wv_sb", name="wv_sb")
        wo_sb = pp.tile([P, MC, D], F16, tag="wo_sb", name="wo_sb")
        xq_sb = pp.tile([P, NQW, KC, 512], F16, tag="xq_sb", name="xq_sb")
        xk_sb = pp.tile([P, NKW, KC, 512], F16, tag="xk_sb", name="xk_sb")
        xv_sb = pp.tile([P, LKC, KC, P], F16, tag="xv_sb", name="xv_sb")
        qt_sb = pp.tile([P, MC, LQ], F16, tag="qt_sb", name="qt_sb")
        kt_sb = pp.tile([P, MC, Lkp], F16, tag="kt_sb", name="kt_sb")
        v_sb = pp.tile([P, LKC, 4, HD + 1], F16, tag="v_sb", name="v_sb")
        ctxt_sb = pp.tile([P, MC, LQ], F16, tag="ctxt_sb", name="ctxt_sb")
        keep_sb = pp.tile([P, LKC], F16, tag="keep_sb", name="keep_sb")

        ones_sb = pp.tile([P, HD], F16, tag="ones_sb", name="ones_sb")

        nc.gpsimd.load_library(library_config.attn)
        nc.vector.memset(ones_sb[0:1, :], 1.0)
        # one queue, hand-ordered by first use: K path, V head, Q window 0
        nc.sync.dma_start(out=wk_sb[:], in_=wk_d.ap())
        nc.sync.dma_start(out=xk_sb[:, 0], in_=xk_d.ap()[0])
        nc.sync.dma_start(out=wq_sb[:], in_=wq_d.ap())
        nc.sync.dma_start(out=xq_sb[:, 0], in_=xq_d.ap()[0])
        for w in range(1, NKW):
            nc.sync.dma_start(out=xk_sb[:, w], in_=xk_d.ap()[w])
        nc.sync.dma_start(out=wv_sb[:], in_=wv_d.ap())
        for lv in range(LKC):
            nc.sync.dma_start(out=xv_sb[:, lv], in_=xv_d.ap()[lv])
        nc.sync.dma_start(out=keep_sb[:], in_=keep_d.ap())
        for w in range(1, NQW):
            nc.sync.dma_start(out=xq_sb[:, w], in_=xq_d.ap()[w])
        nc.sync.dma_start(out=wo_sb[:], in_=wo_d.ap())

        def proj(pool, w_sb, x_sb_w, dst, m):
            """dst[m*128+p, :512] = sum_kc W[kc, m-chunk]^T @ X[kc] (one window)"""
            ps = pool.tile([P, 512], F32, tag="pacc", name="pacc")
            for kc in range(KC):
                nc.tensor.matmul(
                    ps[:],
                    w_sb[:, kc, m * P : (m + 1) * P],
                    x_sb_w[:, kc, :],
                    start=(kc == 0),
                    stop=(kc == KC - 1),
                )
            nc.vector.tensor_copy(dst, ps[:])

        def vproj(pool, lv):
            """v_sb[:, lv] = [Xv[lv] @ Wv^T | keep], key-major with keep col"""
            ps = pool.tile([P, 512], F32, tag="pacc", name="pacc")
            for kc in range(KC):
                nc.tensor.matmul(
                    ps[:, 0:DH],
                    xv_sb[:, lv, kc, :],
                    wv_sb[:, kc, :],
                    start=(kc == 0),
                    stop=(kc == KC - 1),
                )
            nc.vector.tensor_copy(
                v_sb[:, lv, :, 0:HD],
                ps[:, 0:DH].rearrange("p (h c) -> p h c", c=HD),
            )

        with tc.tile_pool(name="pacc", bufs=2, space="PSUM") as pacc_pool, tc.tile_pool(
            name="pss", bufs=2, space="PSUM"
        ) as pss_pool, tc.tile_pool(
            name="psu", bufs=1, space="PSUM"
        ) as psu_pool, tc.tile_pool(
            name="expst", bufs=4
        ) as expst_pool, tc.tile_pool(
            name="uhp", bufs=3
        ) as uh_pool, tc.tile_pool(
            name="smal", bufs=3
        ) as small_pool, tc.tile_pool(
            name="ob", bufs=3
        ) as ob_pool:

            def out_tile(l0):
                """one 128-query row block of the folded output projection"""
                ob = ob_pool.tile([P, D], F16, tag="ob", name="ob_sb")
                for n0 in range(0, D, 512):
                    po = pacc_pool.tile([P, 512], F32, tag="pacc", name="pacc")
                    for m in range(MC):
                        nc.tensor.matmul(
                            po[:],
                            ctxt_sb[:, m, l0 : l0 + P],
                            wo_sb[:, m, n0 : n0 + 512],
                            start=(m == 0),
                            stop=(m == MC - 1),
                        )
                    nc.vector.tensor_copy(ob[:, n0 : n0 + 512], po[:])
                nc.sync.dma_start(out=out_d.ap()[l0 : l0 + P, :], in_=ob[:])

            def qproj_parts(wi, w1, m):
                """Q-projection of one (window, m-chunk), split in two 4-MM
                filler chunks so the PE burst never starves the exp stream"""
                state = {}

                def part1():
                    ps = pacc_pool.tile([P, 512], F32, tag="pacc", name="pacc")
                    state["ps"] = ps
                    for kc in range(KC // 2):
                        nc.tensor.matmul(
                            ps[:],
                            wq_sb[:, kc, m * P : (m + 1) * P],
                            xq_sb[:, wi, kc, :],
                            start=(kc == 0),
                            stop=False,
                        )

                def part2():
                    ps = state["ps"]
                    for kc in range(KC // 2, KC):
                        nc.tensor.matmul(
                            ps[:],
                            wq_sb[:, kc, m * P : (m + 1) * P],
                            xq_sb[:, wi, kc, :],
                            start=False,
                            stop=(kc == KC - 1),
                        )
                    nc.vector.tensor_copy(qt_sb[:, m, w1 : w1 + 512], ps[:])

                return part1, part2

            def attn_half(w0, hp, u, fillers):
                """scores+exp+ctx for one head pair; software-pipelined so the
                PE FIFO never head-of-line blocks on an exp: per step emit
                scores(lk), then U(lk-2), then the step's filler chunks."""
                fillers = dict(fillers)
                expst_t = {}

                def u_mm(lk):
                    ex = expst_t.pop(lk)
                    for hi in range(2):
                        h = 2 * hp + hi
                        # fused ctx+sums: lhsT = [V_h | keep] (M = 65)
                        nc.tensor.matmul(
                            u[0 : HD + 1, hi, :],
                            v_sb[:, lk, h, :],
                            ex[:, hi, :],
                            start=(lk == 0),
                            stop=(lk == LKC - 1),
                        )

                for step in range(LKC + 2):
                    if step < LKC:
                        lk = step
                        ps = pss_pool.tile([P, 2, 512], F32, tag="pss", name="pss_ps")
                        for hi in range(2):
                            b = HD * hi
                            # S^T = K_h @ Q_h^T; the two heads use disjoint
                            # PE row groups and run concurrently
                            nc.tensor.matmul(
                                ps[:, hi, :],
                                kt_sb[b : b + HD, hp, lk * P : (lk + 1) * P],
                                qt_sb[b : b + HD, hp, w0 : w0 + 512],
                                start=True,
                                stop=True,
                                tile_position=(b, 0),
                            )
                        expst = expst_pool.tile(
                            [P, 2, 512], F16, tag="expst", name="expst"
                        )
                        nc.scalar.activation(expst[:], ps[:], EXP, scale=SCALE)
                        expst_t[lk] = expst
                    if step >= 2:
                        u_mm(step - 2)
                    for f in fillers.pop(step, []):
                        f()

            def finish_half(w0, hp, u, cs16, uh_tiles):
                """evacuate U + denominator rows once a head pair completes"""
                uh = uh_pool.tile([P, 2, 512], F16, tag="uh", name="uh_sb")
                uh_tiles.append(uh)
                nc.vector.tensor_copy(uh[0 : HD + 1, :, :], u[0 : HD + 1, :, :])
                eng = nc.sync if hp == 0 else nc.gpsimd
                eng.dma_start(out=cs16[0:1, hp, :, :], in_=uh[HD : HD + 1, :, :])

            def norm_piece(w0, hp, cs16, uh_tiles):
                """one head pair's softmax normalize: PE broadcasts the
                denominators across 64 partitions, reciprocal_approx_fast on
                the broadcast PSUM, then f16 ctx multiplies"""
                rec = small_pool.tile([P, 2, 512], F32, tag="rec", name="rec")
                for hi in range(2):
                    bc = pacc_pool.tile([P, 512], F32, tag="pacc", name="pacc")
                    nc.tensor.matmul(
                        bc[0:HD, :],
                        ones_sb[0:1, :],
                        cs16[0:1, hp, hi, :],
                        start=True,
                        stop=True,
                    )
                    nc.vector.reciprocal_approx_fast(
                        out=rec[0:HD, hi, :], in_=bc[0:HD, :]
                    )
                nc.vector.tensor_mul(
                    ctxt_sb[0:HD, hp, w0 : w0 + 512],
                    uh_tiles[hp][0:HD, 0, :],
                    rec[0:HD, 0, :],
                )
                # odd head lives on partitions 64:128 of the ctx^T chunk;
                # DVE cannot shift partitions: multiply at base 0, move
                # with an SBUF->SBUF DMA
                ct_o = small_pool.tile([P, 512], F16, tag="cto", name="ct_o")
                nc.vector.tensor_mul(
                    ct_o[0:HD, :], uh_tiles[hp][0:HD, 1, :], rec[0:HD, 1, :]
                )
                nc.gpsimd.dma_start(
                    out=ctxt_sb[HD:P, hp, w0 : w0 + 512], in_=ct_o[0:HD, :]
                )

            # HAM warm-up: dummy matmuls while the first DMAs land so the PE
            # clock gate is already released when real work arrives
            dmy = small_pool.tile([P, 640], F16, tag="dmy", name="dmy")
            nc.vector.memset(dmy[:], 0.0)
            pdum = pacc_pool.tile([P, 512], F32, tag="pacc", name="pacc")
            for _ in range(8):
                nc.tensor.matmul(
                    pdum[:], dmy[:, 0:P], dmy[:, P : P + 512], start=True, stop=True
                )

            # prefix: only what window-0 head-pair-0 needs before scoring
            for w in range(NKW):
                proj(pacc_pool, wk_sb, xk_sb[:, w], kt_sb[:, 0, w * 512 : (w + 1) * 512], 0)
            proj(pacc_pool, wq_sb, xq_sb[:, 0], qt_sb[:, 0, 0:512], 0)
            nc.vector.tensor_copy(
                v_sb[:, :, :, HD],
                keep_sb[:, :, None].to_broadcast([P, LKC, 4]),
            )

            prev = None  # (w0, cs16, uh_tiles) of the prior window
            for iw, w0 in enumerate(range(0, LQ, 512)):
                cs16 = small_pool.tile([P, MC, 2, 512], F16, tag="cs16", name="cs16")
                uh_tiles = []
                for hp in range(MC):
                    fillers = {}
                    if iw == 0 and hp == 0:
                        # V-proj streams in exactly two steps ahead of the U
                        # matmul that consumes each block; the m=1 chunks of
                        # K and Q(win0) spread across the same steps
                        for lv in range(LKC):
                            fillers[lv] = [lambda lv=lv: vproj(pacc_pool, lv)]
                        for kw in range(NKW):
                            fillers[1 + 2 * kw].append(lambda kw=kw: proj(
                                pacc_pool, wk_sb, xk_sb[:, kw],
                                kt_sb[:, 1, kw * 512 : (kw + 1) * 512], 1))
                        fillers[5].append(lambda: proj(
                            pacc_pool, wq_sb, xq_sb[:, 0], qt_sb[:, 1, 0:512], 1))
                    elif iw == 0 and hp == 1:
                        fillers[2] = [lambda: proj(
                            pacc_pool, wq_sb, xq_sb[:, 1], qt_sb[:, 0, 512:1024], 0
                        )]
                        fillers[5] = [lambda: proj(
                            pacc_pool, wq_sb, xq_sb[:, 1], qt_sb[:, 1, 512:1024], 1
                        )]
                    else:
                        if hp == 0 and prev is not None:
                            # norm/out-proj of the previous window ride in
                            # late-enough steps not to block this window's
                            # scores in the PE FIFO
                            pw0, pcs, puh = prev
                            fillers[3] = [lambda: norm_piece(pw0, 0, pcs, puh)]
                            fillers[4] = [lambda: norm_piece(pw0, 1, pcs, puh)]
                            for t in range(4):
                                l0 = pw0 + t * P
                                fillers[5 + t] = [lambda l0=l0: out_tile(l0)]
                        if hp == 1:
                            if iw + 1 < NQW:
                                w1 = w0 + 512
                                for m in range(MC):
                                    p1, p2 = qproj_parts(iw + 1, w1, m)
                                    fillers[2 + 4 * m] = [p1]
                                    fillers[4 + 4 * m] = [p2]
                            else:
                                # last window: normalize head-pair 0 early so
                                # only head-pair 1 remains after the last exp
                                fillers[3] = [
                                    lambda: norm_piece(w0, 0, cs16, uh_tiles)
                                ]
                    u = psu_pool.tile([P, 2, 512], F32, tag="u", name="u_ps")
                    attn_half(w0, hp, u, fillers)
                    finish_half(w0, hp, u, cs16, uh_tiles)
                prev = (w0, cs16, uh_tiles)
            pw0, pcs, puh = prev
            norm_piece(pw0, 1, pcs, puh)
            for l0 in range(pw0, pw0 + 512, P):
                out_tile(l0)

    nc.compile()
    nc.m = get_hw_module(nc.m)
    return nc


def _get_nc(Lkp: int):
    if Lkp not in _NC_CACHE:
        _NC_CACHE[Lkp] = _build(Lkp)
    return _NC_CACHE[Lkp]


def _win_layout(x_t, inner):
    """[D, L] -> [L//inner, 128, 8, inner] so each partition's DMA run is contiguous."""
    Ltot = x_t.shape[1]
    return np.ascontiguousarray(
        x_t.reshape(KC, P, Ltot // inner, inner).transpose(2, 1, 0, 3)
    )


def _shard_inputs(query, key, value, mask, Wq, Wk, Wv, Wo):
    B = query.shape[0]
    kept = [np.nonzero(np.asarray(mask[b]) != 0)[0] for b in range(B)]
    lk_max = max((len(k) for k in kept), default=1)
    Lkp = max(512, ((lk_max + 511) // 512) * 512)
    in_maps = []
    for c in range(NCORES):
        b, g = divmod(c, NCORES // B)
        idx = kept[b]
        nk = len(idx)
        xk = np.zeros((D, Lkp), np.float16)
        xv = np.zeros((D, Lkp), np.float16)
        xk[:, :nk] = key[b][idx].T
        xv[:, :nk] = value[b][idx].T
        keepv = np.full((Lkp,), PAD_KEEP, np.float16)
        keepv[:nk] = 1.0
        keepv = np.ascontiguousarray(keepv.reshape(Lkp // P, P).T)
        cols = slice(DH * g, DH * (g + 1))

        def wlay(w):  # [(n p), m] -> [128, n, m]
            return np.ascontiguousarray(
                w.reshape(w.shape[0] // P, P, w.shape[1]).transpose(1, 0, 2).astype(np.float16)
            )

        in_maps.append(
            {
                "xq_t": _win_layout(np.asarray(query[b], np.float32).T.astype(np.float16), 512),
                "xk_t": _win_layout(xk, 512),
                "xv_t": _win_layout(xv, P),
                "keep": keepv,
                "wq_t": wlay(np.asarray(Wq)[cols, :].T.astype(np.float32)),
                "wk_t": wlay(np.asarray(Wk)[cols, :].T.astype(np.float32)),
                "wv_t": wlay(np.asarray(Wv)[cols, :].T.astype(np.float32)),
                "wo_t": wlay(np.asarray(Wo)[:, cols].T.astype(np.float32)),
            }
        )
    return in_maps, Lkp


def kernel(query, key, value, mask, Wq, Wk, Wv, Wo, bo):
    global LAST_RESULTS
    query = np.asarray(query, np.float32)
    key = np.asarray(key, np.float32)
    value = np.asarray(value, np.float32)
    B = query.shape[0]

    in_maps, Lkp = _shard_inputs(query, key, value, mask, Wq, Wk, Wv, Wo)
    nc = _get_nc(Lkp)
    if TRACE:
        _ensure_ntff_hook()
    res = bass_utils.run_bass_kernel_spmd(
        nc, in_maps, list(range(NCORES)), trace=TRACE
    )
    LAST_RESULTS = res

    out = np.zeros((B, LQ, D), np.float32)
    for c in range(NCORES):
        out[c // (NCORES // B)] += res.results[c]["outp"]
    out += np.asarray(bo, np.float32)[None, None, :]
    return out
